# revision 11
# baseline (speedup 1.0000x reference)
"""Cubic B-spline FFD upsampling (stride 5, 42^3 control grid -> 192^3 image).

Full input v: (4, 3, 42, 42, 42) f32 -> output (4, 3, 192, 192, 192) f32.

Each 1D stage is out[o] = sum_c B[o, c] * in[c] with B a banded 192x42 matrix,
so the op is three small matmuls applied along the three spatial axes.

Sharding: 8 cores = batch(4) x first-spatial-axis halves(2). Each core gets
the 23 control-point slices that influence its 96-sample output slab and
computes a (3, 96, 192, 192) slab independently (pure data parallel).

All on-device data is fp16 (tolerance is 2e-2; fp16 pipeline error ~2e-3),
which halves the output-DMA floor vs f32 and keeps PSUM accumulation in f32.

Per-core dataflow (fp16 operands, f32 psum):
  s1 (expand y): lhsT = input slices [42y, (2x, 64z)] (host-padded), rhs =
      B^T [42, 192oy] -> psum [128, 192]; 4 matmuls per 4-bank psum tile,
      2 copies/tile (z-bands at partitions 0/64) -> J[ch] [42z, (48oyq, 4Q, 24x)]
  s2 (expand z): lhsT = J slices [42z, (4Q, 24x)] (Q = oy quarter), rhs =
      B^T [42, 192oz] -> psum [96, 192]; 4 matmuls (4 oyq)/tile, 1 copy
      -> slab [96=(24Q+x), (8 oyq, 192oz)] per (ch, wp) 8-oyq group
  s3 (expand x): lhsT = block-diag Bx [96, 128=(Q,oxr)] per 32-ox block,
      rhs = slab [96, (2 oyq, 192oz)] N=384 -> psum [128=(4Q,32oxr), 384];
      4 matmuls (oyq-pairs)/tile, 1 copy FD=1536 -> bounce -> 4 DMAs (per Q)
      of 295KB with 3KB contiguous DRAM runs.
PSUM->SBUF copies alternate between DVE and ACT by accumulated cost (the
steady-state bottleneck alongside the 21.2MB/core fp16 output DMA).
"""

import numpy as np

import concourse.bass as bass
import concourse.mybir as mybir
import concourse.tile as tile
from concourse import bacc
from concourse.bass_utils import run_bass_kernel_spmd

F32 = mybir.dt.float32
F16 = mybir.dt.float16

N_CORES = 8

_NC_CACHE = None


def _bspline_B() -> np.ndarray:
    """B[o, c]: weight of control point c on cropped output sample o."""
    n = 19
    t = np.abs((np.arange(n) - 9) / 5.0)
    w = np.where(
        t < 1.0,
        2.0 / 3.0 + (0.5 * t - 1.0) * t**2,
        np.where(t < 2.0, -((t - 2.0) ** 3) / 6.0, 0.0),
    )
    o = np.arange(192)[:, None]
    c = np.arange(42)[None, :]
    k = 5 * c + 4 - o
    B = np.where((k >= 0) & (k < 19), w[np.clip(k, 0, 18)], 0.0)
    return np.ascontiguousarray(B, dtype=np.float32)


def _build_nc(reps: int = 1):
    global _NC_CACHE
    if reps == 1 and _NC_CACHE is not None:
        return _NC_CACHE

    nc = bacc.Bacc("TRN2", target_bir_lowering=False, debug=False, num_devices=N_CORES)
    inp = nc.dram_tensor("inp", [42, 4608], F16, kind="ExternalInput").ap()
    byT = nc.dram_tensor("byT", [42, 192], F16, kind="ExternalInput").ap()
    bx3 = nc.dram_tensor("bx3", [96, 384], F16, kind="ExternalInput").ap()
    out = nc.dram_tensor("out", [3, 96, 192, 192], F16, kind="ExternalOutput").ap()

    # out[ch, 32b+oxr, 48Q+8wp+oo, z] viewed as (c wp Q | oxr | b (oo z));
    # (oo z) = 8 consecutive oy rows x 192 oz = one contiguous 3KB run.
    outv = out.rearrange(
        "c (b oxr) (Q wp oo) z -> c wp Q oxr b (oo z)",
        b=3, oxr=32, Q=4, wp=6, oo=8,
    )

    copy_load = [0.0, 0.0]  # accumulated cost on [DVE, ACT]

    def copy(dst, src):
        n = src.free_size()
        cost_dve = n / 0.96 + 130.0
        cost_act = n / 1.2 + 290.0
        if copy_load[0] + cost_dve <= copy_load[1] + cost_act:
            copy_load[0] += cost_dve
            nc.vector.tensor_copy(dst, src)
        else:
            copy_load[1] += cost_act
            nc.scalar.copy(dst, src)

    with tile.TileContext(nc) as tc:
        with (
            tc.tile_pool(name="const", bufs=1) as cpool,
            tc.tile_pool(name="big", bufs=1) as big,
            tc.tile_pool(name="slabs", bufs=3) as slabs,
            tc.tile_pool(name="obp", bufs=3) as obp,
            tc.tile_pool(name="ps", bufs=2, space="PSUM") as psp,
        ):
            tB = cpool.tile([42, 192], F16, name="tB")
            tBx = cpool.tile([96, 384], F16, name="tBx")
            nc.sync.dma_start(tB[:], byT[:])
            nc.sync.dma_start(tBx[:], bx3[:])
            tI = []
            J = []
            for ch in range(3):
                ti = cpool.tile([42, 1536], F16, name=f"tI{ch}")
                nc.sync.dma_start(ti[:], inp[:, 1536 * ch : 1536 * ch + 1536])
                tI.append(ti)
                J.append(big.tile([42, 4608], F16, name=f"J{ch}"))

            for _rep in range(reps):
                # stage 1: expand y. J[ch] free layout = (oyq 48, Q 4, x 24)
                # with oy = 48Q + oyq and x = 2*xp + e (parity e in psum
                # partition bands 0/64), so stage-2 lhsT windows [96q:96q+96]
                # are contiguous (the stationary AP must be one free dim).
                for ch in range(3):
                    Jw = J[ch].rearrange(
                        "p (oyq Q xp e) -> p e xp Q oyq", oyq=48, Q=4, xp=12, e=2
                    )
                    for t3 in range(3):
                        ps = psp.tile([128, 2048], F32, name="ps1", tag="ps")
                        for i in range(4):
                            p = 4 * t3 + i
                            nc.tensor.matmul(
                                ps[:, 512 * i : 512 * i + 192],
                                tI[ch][:, 128 * p : 128 * p + 128],
                                tB[:],
                                start=True, stop=True,
                            )
                        ps4 = ps.rearrange("p (i n) -> p i n", i=4)[
                            :, :, 0:192
                        ].rearrange("p i (Q oyq) -> p i Q oyq", Q=4)
                        copy(Jw[0:42, 0, 4 * t3 : 4 * t3 + 4], ps4[0:42])
                        copy(Jw[0:42, 1, 4 * t3 : 4 * t3 + 4], ps4[64:106])

                # stages 2+3 interleaved per (ch, wp): produce an 8-oyq slab,
                # immediately expand x and stream out.
                for ch in range(3):
                    for wp in range(6):
                        slab = slabs.tile([96, 1536], F16, name="slab")
                        slr = slab.rearrange("p (q z) -> p q z", q=8)
                        for k in range(2):
                            ps = psp.tile([128, 2048], F32, name="ps2", tag="ps")
                            for i in range(4):
                                q = 8 * wp + 4 * k + i
                                nc.tensor.matmul(
                                    ps[0:96, 512 * i : 512 * i + 192],
                                    J[ch][:, 96 * q : 96 * q + 96],
                                    tB[:],
                                    start=True, stop=True,
                                )
                            ps4 = ps.rearrange("p (i n) -> p i n", i=4)
                            copy(slr[0:96, 4 * k : 4 * k + 4, :], ps4[0:96, :, 0:192])
                        # stage 3: bounce layout (b 3, ul 4, 384) — b-major so
                        # each b-block is one contiguous (8 oy x 192 oz) run
                        ob = obp.tile([128, 4608], F16, name="ob")
                        obr = ob.rearrange("p (b ul n) -> p b ul n", b=3, ul=4)
                        for b in range(3):
                            ps = psp.tile([128, 2048], F32, name="ps3", tag="ps")
                            for ul in range(4):
                                nc.tensor.matmul(
                                    ps[:, 512 * ul : 512 * ul + 384],
                                    tBx[:, 128 * b : 128 * b + 128],
                                    slr[0:96, 2 * ul : 2 * ul + 2, :],
                                    start=True, stop=True,
                                )
                            ps4 = ps.rearrange("p (i n) -> p i n", i=4)
                            copy(obr[:, b, :, :], ps4[:, :, 0:384])
                        obd = ob.rearrange("p (b m) -> p b m", b=3)
                        for Q in range(4):
                            nc.sync.dma_start(
                                outv[ch, wp, Q], obd[32 * Q : 32 * Q + 32]
                            )
    nc.compile()
    if reps == 1:
        _NC_CACHE = nc
    return nc


def make_inputs(v: np.ndarray) -> list[dict[str, np.ndarray]]:
    """Per-core input maps from the full (4, 3, 42, 42, 42) tensor."""
    B = _bspline_B()
    byT = np.ascontiguousarray(B.T).astype(np.float16)  # (42, 192)
    ins = []
    for core in range(N_CORES):
        b, h = divmod(core, 2)
        c0 = 19 * h
        vs = v[b, :, c0 : c0 + 23, :, :].astype(np.float16)  # (ch, x, y, z)
        vt = np.transpose(vs, (2, 0, 1, 3))  # (y, ch, x, z)
        ti = np.zeros((42, 3, 12, 2, 64), np.float16)
        ti.reshape(42, 3, 24, 64)[:, :, :23, :42] = vt
        Bxh = B[96 * h : 96 * h + 96, c0 : c0 + 23].astype(np.float16)  # (96ox, 23x)
        bx3 = np.zeros((4, 24, 3, 4, 32), np.float16)  # (Q, x, b, Q', oxr)
        for Q in range(4):
            for blk in range(3):
                bx3[Q, :23, blk, Q, :] = Bxh[32 * blk : 32 * blk + 32, :].T
        ins.append({
            "inp": np.ascontiguousarray(ti.reshape(42, 4608)),
            "byT": byT,
            "bx3": np.ascontiguousarray(bx3.reshape(96, 384)),
        })
    return ins


def assemble(results: list[dict[str, np.ndarray]]) -> np.ndarray:
    full = np.empty((4, 3, 192, 192, 192), np.float32)
    for core in range(N_CORES):
        b, h = divmod(core, 2)
        full[b, :, 96 * h : 96 * h + 96, :, :] = results[core]["out"].astype(
            np.float32
        )
    return full


def kernel(v: np.ndarray) -> np.ndarray:
    v = np.ascontiguousarray(np.asarray(v, dtype=np.float32))
    assert v.shape == (4, 3, 42, 42, 42)
    nc = _build_nc()
    ins = make_inputs(v)
    res = run_bass_kernel_spmd(nc, ins, list(range(N_CORES)))
    return assemble(res.results)


# revision 18
# speedup vs baseline: 1.6203x; 1.6203x over previous
"""Cubic B-spline FFD upsampling (stride 5, 42^3 control grid -> 192^3 image).

Full input v: (4, 3, 42, 42, 42) f32 -> output (4, 3, 192, 192, 192) f32.

Each 1D stage is out[o] = sum_c B[o, c] * in[c] with B a banded 192x42 matrix,
so the op is three small matmuls applied along the three spatial axes.

Sharding: 8 cores = batch(4) x first-spatial-axis halves(2). Each core gets
the 23 control-point slices that influence its 96-sample output slab and
computes a (3, 96, 192, 192) slab independently (pure data parallel).

All on-device data is fp16 (tolerance is 2e-2; fp16 pipeline error ~2e-3),
which halves the output-DMA floor vs f32 and keeps PSUM accumulation in f32.

Per-core dataflow (fp16 operands, f32 psum):
  s1 (expand y): lhsT = input slices [42y, (2x, 64z)] (host-padded), rhs =
      B^T [42, 192oy] -> psum [128, 192]; 4 matmuls per 4-bank psum tile,
      2 copies/tile (z-bands at partitions 0/64) -> J[ch] [42z, (48oyq, 4Q, 24x)]
  s2 (expand z): lhsT = J slices [42z, (4Q, 24x)] (Q = oy quarter), rhs =
      B^T [42, 192oz] -> psum [96, 192]; 4 matmuls (4 oyq)/tile, 1 copy
      -> slab [96=(24Q+x), (8 oyq, 192oz)] per (ch, wp) 8-oyq group
  s3 (expand x): lhsT = block-diag Bx [96, 128=(Q,oxr)] per 32-ox block,
      rhs = slab [96, (2 oyq, 192oz)] N=384 -> psum [128=(4Q,32oxr), 384];
      4 matmuls (oyq-pairs)/tile, 1 copy FD=1536 -> bounce -> 4 DMAs (per Q)
      of 295KB with 3KB contiguous DRAM runs.
PSUM->SBUF copies alternate between DVE and ACT by accumulated cost (the
steady-state bottleneck alongside the 21.2MB/core fp16 output DMA).
"""

import numpy as np

import concourse.bass as bass
import concourse.mybir as mybir
import concourse.tile as tile
from concourse import bacc
from concourse.bass_utils import run_bass_kernel_spmd

F32 = mybir.dt.float32
F16 = mybir.dt.float16

N_CORES = 8

_NC_CACHE = None


def _bspline_B() -> np.ndarray:
    """B[o, c]: weight of control point c on cropped output sample o."""
    n = 19
    t = np.abs((np.arange(n) - 9) / 5.0)
    w = np.where(
        t < 1.0,
        2.0 / 3.0 + (0.5 * t - 1.0) * t**2,
        np.where(t < 2.0, -((t - 2.0) ** 3) / 6.0, 0.0),
    )
    o = np.arange(192)[:, None]
    c = np.arange(42)[None, :]
    k = 5 * c + 4 - o
    B = np.where((k >= 0) & (k < 19), w[np.clip(k, 0, 18)], 0.0)
    return np.ascontiguousarray(B, dtype=np.float32)


def _build_nc(reps: int = 1):
    global _NC_CACHE
    if reps == 1 and _NC_CACHE is not None:
        return _NC_CACHE

    nc = bacc.Bacc("TRN2", target_bir_lowering=False, debug=False, num_devices=N_CORES)
    inp = nc.dram_tensor("inp", [42, 4608], F16, kind="ExternalInput").ap()
    byT = nc.dram_tensor("byT", [42, 192], F16, kind="ExternalInput").ap()
    bx3 = nc.dram_tensor("bx3", [96, 384], F16, kind="ExternalInput").ap()
    out = nc.dram_tensor("out", [3, 96, 192, 192], F16, kind="ExternalOutput").ap()

    # out[ch, 32b+oxr, 48Q+8wp+oo, z] viewed as (c wp Q | oxr | b (oo z));
    # (oo z) = 8 consecutive oy rows x 192 oz = one contiguous 3KB run.
    outv = out.rearrange(
        "c (b oxr) (Q wp oo) z -> c wp Q oxr b (oo z)",
        b=3, oxr=32, Q=4, wp=6, oo=8,
    )

    copy_load = [0.0, 0.0]  # accumulated cost on [DVE, ACT]

    def copy(dst, src):
        n = src.free_size()
        cost_dve = (n + 120.0) / 0.96
        cost_act = (n + 222.0) / 1.2
        if copy_load[0] + cost_dve <= copy_load[1] + cost_act:
            copy_load[0] += cost_dve
            nc.vector.tensor_copy(dst, src)
        else:
            copy_load[1] += cost_act
            nc.scalar.copy(dst, src)

    with tile.TileContext(nc) as tc:
        with (
            tc.tile_pool(name="const", bufs=1) as cpool,
            tc.tile_pool(name="big", bufs=1) as big,
            tc.tile_pool(name="slabs", bufs=6) as slabs,
            tc.tile_pool(name="obp", bufs=8) as obp,
            tc.tile_pool(name="ps", bufs=4, space="PSUM") as psp,
        ):
            tB = cpool.tile([42, 192], F16, name="tB")
            tBx = cpool.tile([96, 384], F16, name="tBx")
            nc.sync.dma_start(tB[:], byT[:])
            nc.sync.dma_start(tBx[:], bx3[:])
            tI = []
            J = []
            for ch in range(3):
                ti = cpool.tile([42, 1536], F16, name=f"tI{ch}")
                nc.sync.dma_start(ti[:], inp[:, 1536 * ch : 1536 * ch + 1536])
                tI.append(ti)
                J.append(big.tile([42, 4608], F16, name=f"J{ch}"))

            for _rep in range(reps):
                # stage 1: expand y. J[ch] free layout = (oyq 48, Q 4, x 24)
                # with oy = 48Q + oyq and x = 2*xp + e (parity e in psum
                # partition bands 0/64), so stage-2 lhsT windows [96q:96q+96]
                # are contiguous (the stationary AP must be one free dim).
                for ch in range(3):
                    Jw = J[ch].rearrange(
                        "p (oyq Q xp e) -> p e xp Q oyq", oyq=48, Q=4, xp=12, e=2
                    )
                    for t6 in range(6):
                        ps = psp.tile([128, 1024], F32, name="ps1", tag="ps")
                        for i in range(2):
                            p = 2 * t6 + i
                            nc.tensor.matmul(
                                ps[:, 512 * i : 512 * i + 192],
                                tI[ch][:, 128 * p : 128 * p + 128],
                                tB[:],
                                start=True, stop=True,
                            )
                        ps4 = ps.rearrange("p (i n) -> p i n", i=2)[
                            :, :, 0:192
                        ].rearrange("p i (Q oyq) -> p i Q oyq", Q=4)
                        copy(Jw[0:42, 0, 2 * t6 : 2 * t6 + 2], ps4[0:42])
                        copy(Jw[0:42, 1, 2 * t6 : 2 * t6 + 2], ps4[64:106])

                # stages 2+3 interleaved per (ch, wp): produce an 8-oyq slab,
                # immediately expand x and stream out.
                def fill_s2(ch, wp):
                    # stage 2: one 8-oyq slab for (ch, wp)
                    slab = slabs.tile([96, 1536], F16, name="slab")
                    slr = slab.rearrange("p (q z) -> p q z", q=8)
                    for k in range(2):
                        ps = psp.tile([128, 1024], F32, name="ps2", tag="ps")
                        for i in range(4):
                            q = 8 * wp + 4 * k + i
                            off = 512 * (i // 2) + 192 * (i % 2)
                            nc.tensor.matmul(
                                ps[0:96, off : off + 192],
                                J[ch][:, 96 * q : 96 * q + 96],
                                tB[:],
                                start=True, stop=True,
                            )
                        ps4 = ps.rearrange("p (bk n) -> p bk n", bk=2)[
                            :, :, 0:384
                        ].rearrange("p bk (j z) -> p bk j z", j=2)
                        copy(slr[0:96, 4 * k : 4 * k + 4, :].rearrange(
                            "p (bk j) z -> p bk j z", bk=2), ps4[0:96])
                    return slr

                def do_s3(ch, wp, slr):
                    # stage 3: bounce layout (b 3, ul 4, 384) — b-major so
                    # each b-block is one contiguous (8 oy x 192 oz) run
                    ob = obp.tile([128, 4608], F16, name="ob")
                    obr = ob.rearrange("p (b ul n) -> p b ul n", b=3, ul=4)
                    for b in range(3):
                        for half in range(2):
                            ps = psp.tile([128, 1024], F32, name="ps3", tag="ps")
                            for i in range(2):
                                ul = 2 * half + i
                                nc.tensor.matmul(
                                    ps[:, 512 * i : 512 * i + 384],
                                    tBx[:, 128 * b : 128 * b + 128],
                                    slr[0:96, 2 * ul : 2 * ul + 2, :],
                                    start=True, stop=True,
                                )
                            ps4 = ps.rearrange("p (i n) -> p i n", i=2)
                            copy(
                                obr[:, b, 2 * half : 2 * half + 2, :],
                                ps4[:, :, 0:384],
                            )
                    obd = ob.rearrange("p (b m) -> p b m", b=3)
                    for Q in range(4):
                        nc.sync.dma_start(outv[ch, wp, Q], obd[32 * Q : 32 * Q + 32])

                # software pipeline: issue s2 of group g+1 before s3 of
                # group g so the copy engines always have ready work
                prev = None
                for ch in range(3):
                    for wp in range(6):
                        slr = fill_s2(ch, wp)
                        if prev is not None:
                            do_s3(*prev)
                        prev = (ch, wp, slr)
                do_s3(*prev)
    nc.compile()
    if reps == 1:
        _NC_CACHE = nc
    return nc


def make_inputs(v: np.ndarray) -> list[dict[str, np.ndarray]]:
    """Per-core input maps from the full (4, 3, 42, 42, 42) tensor."""
    B = _bspline_B()
    byT = np.ascontiguousarray(B.T).astype(np.float16)  # (42, 192)
    ins = []
    for core in range(N_CORES):
        b, h = divmod(core, 2)
        c0 = 19 * h
        vs = v[b, :, c0 : c0 + 23, :, :].astype(np.float16)  # (ch, x, y, z)
        vt = np.transpose(vs, (2, 0, 1, 3))  # (y, ch, x, z)
        ti = np.zeros((42, 3, 12, 2, 64), np.float16)
        ti.reshape(42, 3, 24, 64)[:, :, :23, :42] = vt
        Bxh = B[96 * h : 96 * h + 96, c0 : c0 + 23].astype(np.float16)  # (96ox, 23x)
        bx3 = np.zeros((4, 24, 3, 4, 32), np.float16)  # (Q, x, b, Q', oxr)
        for Q in range(4):
            for blk in range(3):
                bx3[Q, :23, blk, Q, :] = Bxh[32 * blk : 32 * blk + 32, :].T
        ins.append({
            "inp": np.ascontiguousarray(ti.reshape(42, 4608)),
            "byT": byT,
            "bx3": np.ascontiguousarray(bx3.reshape(96, 384)),
        })
    return ins


def assemble(results: list[dict[str, np.ndarray]]) -> np.ndarray:
    full = np.empty((4, 3, 192, 192, 192), np.float32)
    for core in range(N_CORES):
        b, h = divmod(core, 2)
        full[b, :, 96 * h : 96 * h + 96, :, :] = results[core]["out"].astype(
            np.float32
        )
    return full


def kernel(v: np.ndarray) -> np.ndarray:
    v = np.ascontiguousarray(np.asarray(v, dtype=np.float32))
    assert v.shape == (4, 3, 42, 42, 42)
    nc = _build_nc()
    ins = make_inputs(v)
    res = run_bass_kernel_spmd(nc, ins, list(range(N_CORES)))
    return assemble(res.results)
